# revision 37
# baseline (speedup 1.0000x reference)
"""Trainium2 Bass kernel for nn_ModelClass_45724221833594 (gnn_message_passing).

Data-parallel across graphs: 64 graphs x 1024 nodes x 64 feats, kNN(k=6),
pre_nn MLP + global BatchNorm, 4x GeneralConv (add-aggr) + JK-cat pooling,
per-graph FFN. 8 NeuronCores, 8 graphs per core; only cross-core traffic is
a [64,2] AllReduce for the BatchNorm statistics.

v2: f16 everywhere on the tensor engine (fp32 runs at 1/4 rate), host-side
x transpose+cast (kills PE transposes), K=65 scores with mean-shifted
norm row (ranking-invariant), psum-direct top-8, per-graph-pair
score/conv software pipeline to keep all engines busy.

Self-contained: hardcodes all shapes; host side only slices/reshapes inputs.
"""

import sys

for _p in ("/opt/trn_rl_repo", "/root/.axon_site/_ro/trn_rl_repo"):
    if _p not in sys.path:
        sys.path.append(_p)

from contextlib import ExitStack

import numpy as np

import concourse.bass as bass
import concourse.tile as tile
from concourse import bacc, mybir
from concourse import bass_utils

F32 = mybir.dt.float32
F16 = mybir.dt.float16
U16 = mybir.dt.uint16
I16 = mybir.dt.int16
AF = mybir.ActivationFunctionType
OP = mybir.AluOpType

G, P, D, K = 64, 1024, 64, 6
NC = 8           # cores
GPC = G // NC    # graphs per core = 8
NPC = GPC * P    # nodes per core = 8192
CH = 512         # node chunk
NCH = NPC // CH  # 16 chunks per core
NTOT = G * P     # total nodes (for BN)


def build_program(num_cores=NC, sim_single=False):
    nc = bacc.Bacc("TRN2", target_bir_lowering=False, debug=False, num_devices=num_cores)

    din = {}
    def inp(name, shape, dtype=F32):
        din[name] = nc.dram_tensor(name, list(shape), dtype, kind="ExternalInput").ap()
        return din[name]

    x16_d = inp("x16", [D, NPC], F16)     # x^T, host-transposed + f16
    w1b_d = inp("w1b", [D + 1, D], F16)
    w2b_d = inp("w2b", [D + 1, D], F16)
    al_d = inp("alphas", [D, 6])          # a1, 1-a1, a2, 1-a2, aact, -aact (f32)
    gb_d = inp("gb", [D, 2])              # gamma, beta (f32)
    cw_d = inp("convw", [D, 4 * D], F16)  # W_l at cols l*64
    cb_d = inp("i64b", [D + 1, 4 * D], F16)  # per-layer [I64; 6*conv_b[l]]
    sqw_d = inp("sqw", [D + 1, 1], F16)   # rows 0-63 = -0.5, row 64 = +32
    fw1_d = inp("fw1", [D, 5 * 320])      # ffn_W1 [320,320] -> [64, L*320+j] (f32)
    fb1_d = inp("fb1", [1, 320])
    fw2_d = inp("fw2", [128, 3])          # ffn_W2 padded per 128-chunk
    fb2_d = inp("fb2", [1, 1])
    ones_d = inp("ones16", [1, NPC], F16)  # ones row (fast row init via DMA)

    z_d = nc.dram_tensor("zout", [1, GPC], F32, kind="ExternalOutput").ap()

    ccin = nc.dram_tensor("ccin", [D, 2], F32).ap()
    ccout = nc.dram_tensor("ccout", [D, 2], F32, addr_space="Shared").ap()

    with tile.TileContext(nc) as tc, ExitStack() as ctx:
        pool = lambda name, bufs: ctx.enter_context(tc.tile_pool(name=name, bufs=bufs))
        cpool = pool("consts", 1)
        ppoolA = ctx.enter_context(tc.tile_pool(name="pA", bufs=3, space="PSUM"))
        ppoolB = ctx.enter_context(tc.tile_pool(name="pB", bufs=1, space="PSUM"))
        spool = ctx.enter_context(tc.tile_pool(name="psco", bufs=2, space="PSUM"))

        # ---- constants to SBUF ----
        w1b = cpool.tile([D + 1, D], F16); nc.sync.dma_start(w1b[:], w1b_d[:])
        w2b = cpool.tile([D + 1, D], F16); nc.sync.dma_start(w2b[:], w2b_d[:])
        al = cpool.tile([D, 6], F32); nc.sync.dma_start(al[:], al_d[:])
        gb = cpool.tile([D, 2], F32); nc.sync.dma_start(gb[:], gb_d[:])
        cw = cpool.tile([D, 4 * D], F16); nc.sync.dma_start(cw[:], cw_d[:])
        i64b = cpool.tile([D + 1, 4 * D], F16); nc.sync.dma_start(i64b[:], cb_d[:])
        sqw = cpool.tile([D + 1, 1], F16); nc.sync.dma_start(sqw[:], sqw_d[:])
        fw1 = cpool.tile([D, 5 * 320], F32); nc.sync.dma_start(fw1[:], fw1_d[:])
        fb1 = cpool.tile([1, 320], F32); nc.sync.dma_start(fb1[:], fb1_d[:])
        fw2 = cpool.tile([128, 3], F32); nc.sync.dma_start(fw2[:], fw2_d[:])
        fb2 = cpool.tile([1, 1], F32); nc.sync.dma_start(fb2[:], fb2_d[:])
        ones6 = cpool.tile([128, K], F16); nc.vector.memset(ones6[:], 1.0)
        ones512 = cpool.tile([1, CH], F32); nc.vector.memset(ones512[:], 1.0)

        a1, a1m = al[:, 0:1], al[:, 1:2]
        a2, a2m = al[:, 2:3], al[:, 3:4]
        aact, naact = al[:, 4:5], al[:, 5:6]
        gamma, beta = gb[:, 0:1], gb[:, 1:2]

        # ---- big SBUF residents ----
        # X1: rows 0-63 x^T (f16), row 64 = 32 - |x|^2/2  (scores rhs)
        # X2: rows 0-63 x^T (f16), row 64 = ones          (scores lhsT, pre_nn rhs)
        X1 = cpool.tile([D + 1, NPC], F16)
        X2 = cpool.tile([D + 1, NPC], F16)
        nc.sync.dma_start(X2[D : D + 1, :], ones_d[:])
        h16 = cpool.tile([D + 1, NPC], F16)  # evolving h (f16); row 64 ones
        nc.sync.dma_start(h16[D : D + 1, :], ones_d[:])

        # accumulators for pooling (5 layers x 16 chunks) and BN stats
        pacc = cpool.tile([D, 5 * NCH], F32)
        hsum = cpool.tile([D, NCH], F32)
        hsq = cpool.tile([D, NCH], F32)

        # rotating f16 staging tiles with a preset ones row (K=65 bias trick)
        xsqs = []
        for _xi in range(3):
            _t = cpool.tile([D + 1, CH], F16, tag=f"xsq{_xi}")
            nc.sync.dma_start(_t[D : D + 1, :], ones_d[:, 0:CH])
            xsqs.append(_t)
        h1cs = []
        for _xi in range(3):
            _t = cpool.tile([D + 1, CH], F16, tag=f"h1c{_xi}")
            nc.sync.dma_start(_t[D : D + 1, :], ones_d[:, 0:CH])
            h1cs.append(_t)
        sqd = cpool.tile([D, CH], F16)  # scratch dst for BN sum-of-squares

        mv_p = pool("maxv", 4)
        adj_p = pool("adj", 1)
        adjt_p = pool("adjt", 6)
        u16_p = pool("u16", 2)
        sm_p = pool("small", 1)

        # DMA x^T into X1/X2 feature rows (4 chunks each for overlap)
        for q in range(4):
            qs = slice(q * 2048, (q + 1) * 2048)
            nc.sync.dma_start(X1[0:D, qs], x16_d[:, qs])
            nc.sync.dma_start(X2[0:D, qs], x16_d[:, qs])

        # ========= Phase 1+2 interleaved: norm rows, pre_nn, BN stats =========
        for c in range(NCH):
            sl = slice(c * CH, (c + 1) * CH)
            xsq = xsqs[c % 3]
            nc.vector.tensor_tensor(xsq[0:D, :], X1[0:D, sl], X1[0:D, sl], op=OP.mult)
            psq = ppoolA.tile([1, CH], F32, tag="pA")
            nc.tensor.matmul(psq[:], sqw[:], xsq[:])  # 32 - |x|^2/2
            nc.scalar.copy(X1[D : D + 1, sl], psq[:])
            p1 = ppoolA.tile([D, CH], F32, tag="pA")
            nc.tensor.matmul(p1[:], w1b[:], X2[:, sl])  # K=65 (bias via ones row)
            h1c = h1cs[c % 3]
            nc.scalar.activation(h1c[0:D, :], p1[:], AF.Prelu, alpha=a1)
            p2 = ppoolA.tile([D, CH], F32, tag="pA")
            nc.tensor.matmul(p2[:], w2b[:], h1c[:])  # K=65
            nc.scalar.activation(
                h16[0:D, sl], p2[:], AF.Prelu, alpha=a2,
                accum_out=hsum[:, c : c + 1],
            )
            # sum of squares for BN var (vector, to keep the scalar queue short)
            nc.vector.tensor_tensor(sqd[:], h16[0:D, sl], h16[0:D, sl], op=OP.mult)
            nc.vector.reduce_sum(hsq[:, c : c + 1], sqd[:], axis=mybir.AxisListType.X)

        # BN statistics allreduce (kick off early; math happens after scores g0/g1)
        st = sm_p.tile([D, 16], F32, tag="stats")
        nc.vector.reduce_sum(st[:, 0:1], hsum[:], axis=mybir.AxisListType.X)
        nc.vector.reduce_sum(st[:, 1:2], hsq[:], axis=mybir.AxisListType.X)
        ccr = st[:, 2:4]
        if sim_single:
            nc.scalar.copy(ccr, st[:, 0:2])
        else:
            nc.scalar.dma_start(ccin[:], st[:, 0:2])
            nc.gpsimd.collective_compute(
                "AllReduce",
                OP.add,
                replica_groups=[list(range(num_cores))],
                ins=[ccin[:]],
                outs=[ccout[:]],
            )
            nc.scalar.dma_start(ccr, ccout[:])

        # ================= kNN scores + top-8 + adjacency =================
        adjts = [None] * GPC

        def scores_graph(g):
            adj = adj_p.tile([128, GPC, P], F16)
            for b in range(8):  # query blocks of 128
                qsl = slice(g * P + b * 128, g * P + (b + 1) * 128)
                ps = spool.tile([128, P], F32, tag="psco")
                for hf in range(2):
                    csl = slice(g * P + hf * CH, g * P + (hf + 1) * CH)
                    nc.tensor.matmul(ps[:, hf * CH : (hf + 1) * CH], X2[:, qsl], X1[:, csl])
                vals = mv_p.tile([128, 8], F32, tag="mvals")
                idx = mv_p.tile([128, 8], U16, tag="midx")
                nc.vector.max(vals[:], ps[:])
                nc.vector.max_index(idx[:], vals[:], ps[:])
                nc.gpsimd.local_scatter(
                    adj[:, b, :],
                    ones6[:],
                    idx[:, 1 : K + 1].bitcast(I16),
                    channels=128,
                    num_elems=P,
                    num_idxs=K,
                )
            adjt = adjt_p.tile([128, 64, 128], F16)
            nc.sync.dma_start_transpose(adjt[:], adj[:])
            adjts[g] = adjt

        for _g in range(6):
            scores_graph(_g)

        # ================= BN stats math + apply =================
        mean, msq = st[:, 4:5], st[:, 5:6]
        var, rv, rstd = st[:, 6:7], st[:, 7:8], st[:, 8:9]
        scale, shift, tmp = st[:, 9:10], st[:, 10:11], st[:, 11:12]
        ntot = NPC if sim_single else NTOT
        nc.vector.tensor_scalar_mul(mean, st[:, 2:3], 1.0 / ntot)
        nc.vector.tensor_scalar_mul(msq, st[:, 3:4], 1.0 / ntot)
        nc.vector.tensor_tensor(tmp, mean, mean, op=OP.mult)
        nc.vector.tensor_tensor(var, msq, tmp, op=OP.subtract)
        nc.vector.tensor_scalar_add(rv, var, 1e-5)
        nc.vector.reciprocal(rv, rv)
        nc.scalar.sqrt(rstd, rv)  # rstd = sqrt(1/(var+eps))
        nc.vector.tensor_tensor(scale, gamma, rstd, op=OP.mult)
        nc.vector.tensor_tensor(tmp, mean, scale, op=OP.mult)
        nc.vector.tensor_tensor(shift, beta, tmp, op=OP.subtract)
        for c in range(NCH):
            sl = slice(c * CH, (c + 1) * CH)
            nc.scalar.activation(
                h16[0:D, sl], h16[0:D, sl], AF.Identity,
                bias=shift, scale=scale, accum_out=pacc[:, c : c + 1],
            )

        # ================= conv layers =================
        def conv_graph_layer(g, l):
            av = adjts[g][:].rearrange("p (qb cb) f -> p cb qb f", cb=8)
            wl = cw[:, l * D : (l + 1) * D]
            pu = ppoolA.tile([128, 8 * D], F32, tag="pA")
            for sc in range(8):
                nc.tensor.matmul(
                    pu[:, sc * D : (sc + 1) * D],
                    h16[0:D, g * P + sc * 128 : g * P + (sc + 1) * 128],
                    wl,
                )
            u16 = u16_p.tile([128, GPC * D], F16)
            nc.scalar.copy(u16[:], pu[:])
            for dc in range(2):
                dsl = slice(g * P + dc * CH, g * P + (dc + 1) * CH)
                pm = ppoolA.tile([D, CH], F32, tag="pA")
                for sc in range(8):
                    nc.tensor.matmul(
                        pm[:],
                        u16[:, sc * D : (sc + 1) * D],
                        av[:, sc, 4 * dc : 4 * dc + 4, :],
                        start=(sc == 0),
                        stop=False,
                        skip_group_check=True,
                    )
                nc.tensor.matmul(
                    pm[:], i64b[:, l * D : (l + 1) * D], h16[:, dsl],
                    start=False, stop=True, skip_group_check=True,
                )
                col = (l + 1) * NCH + g * 2 + dc
                nc.scalar.activation(
                    h16[0:D, dsl], pm[:], AF.Prelu, alpha=aact,
                    accum_out=pacc[:, col : col + 1],
                )

        for gp in range(0, GPC, 2):
            if gp + 6 < GPC:
                scores_graph(gp + 6)
                scores_graph(gp + 7)
            for l in range(4):
                conv_graph_layer(gp, l)
                conv_graph_layer(gp + 1, l)

        # ================= pooling + FFN =================
        pooledT = sm_p.tile([D, 5 * GPC], F32, tag="pooled")  # col = L*8+g
        nc.vector.tensor_tensor(
            pooledT[:],
            pacc[:].rearrange("p (n two) -> p n two", two=2)[:, :, 0],
            pacc[:].rearrange("p (n two) -> p n two", two=2)[:, :, 1],
            op=OP.add,
        )
        z1L = sm_p.tile([128, 3 * GPC], F32, tag="z1l")
        for mc in range(3):
            msz = min(128, 320 - mc * 128)
            pz = ppoolB.tile([128, GPC], F32, tag="pB")
            for L in range(5):
                nc.tensor.matmul(
                    pz[0:msz, :],
                    fw1[:, L * 320 + mc * 128 : L * 320 + mc * 128 + msz],
                    pooledT[:, L * GPC : (L + 1) * GPC],
                    start=(L == 0),
                    stop=False,
                )
            nc.tensor.matmul(
                pz[0:msz, :], fb1[:, mc * 128 : mc * 128 + msz], ones512[:, 0:GPC],
                start=False, stop=True,
            )
            # LeakyReLU(0.01)
            nc.scalar.activation(
                z1L[0:msz, mc * GPC : (mc + 1) * GPC], pz[0:msz, :], AF.Prelu,
                alpha=0.01,
            )
        pzz = ppoolB.tile([1, GPC], F32, tag="pB")
        for mc in range(3):
            msz = min(128, 320 - mc * 128)
            nc.tensor.matmul(
                pzz[:], fw2[0:msz, mc : mc + 1], z1L[0:msz, mc * GPC : (mc + 1) * GPC],
                start=(mc == 0), stop=False,
            )
        nc.tensor.matmul(pzz[:], fb2[:], ones512[:, 0:GPC], start=False, stop=True)
        zsb = sm_p.tile([1, GPC], F32, tag="zsb")
        nc.scalar.copy(zsb[:], pzz[:])
        nc.sync.dma_start(z_d[:], zsb[:])

    nc.compile()
    return nc


def make_in_maps(inputs):
    """Host-side marshalling: slice + transpose x per core, lay out weights."""
    x = np.asarray(inputs["x"], dtype=np.float32)
    f32 = lambda a: np.asarray(a, dtype=np.float32)
    f16c = lambda a: np.ascontiguousarray(np.asarray(a, dtype=np.float16))

    w1b = np.concatenate([f32(inputs["pre_W1"]), f32(inputs["pre_b1"])[None, :]], 0)
    w2b = np.concatenate([f32(inputs["pre_W2"]), f32(inputs["pre_b2"])[None, :]], 0)
    a1 = f32(inputs["pre_a1"]); a2 = f32(inputs["pre_a2"]); aa = f32(inputs["act_a"])
    alphas = np.stack([a1, 1.0 - a1, a2, 1.0 - a2, aa, -aa], 1)
    gb = np.stack([f32(inputs["bn_gamma"]), f32(inputs["bn_beta"])], 1)
    convw = f32(inputs["conv_W"]).transpose(1, 0, 2).reshape(D, 4 * D)
    i64b = np.zeros((D + 1, 4 * D), np.float32)
    for l in range(4):
        i64b[0:D, l * D : (l + 1) * D] = np.eye(D, dtype=np.float32)
        i64b[D, l * D : (l + 1) * D] = 6.0 * f32(inputs["conv_b"])[l]
    fw1 = f32(inputs["ffn_W1"]).reshape(5, D, 320).transpose(1, 0, 2).reshape(D, 5 * 320)
    fb1 = f32(inputs["ffn_b1"]).reshape(1, 320)
    fw2 = np.zeros((128, 3), np.float32)
    w2flat = f32(inputs["ffn_W2"]).reshape(320)
    for mc in range(3):
        msz = min(128, 320 - mc * 128)
        fw2[:msz, mc] = w2flat[mc * 128 : mc * 128 + msz]
    fb2 = f32(inputs["ffn_b2"]).reshape(1, 1)
    sqw = np.zeros((D + 1, 1), np.float32)
    sqw[0:D, 0] = -0.5
    sqw[D, 0] = 32.0

    shared = {
        "ones16": np.ones((1, NPC), np.float16),
        "sqw": f16c(sqw),
        "w1b": f16c(w1b), "w2b": f16c(w2b),
        "alphas": np.ascontiguousarray(alphas), "gb": np.ascontiguousarray(gb),
        "convw": f16c(convw), "i64b": f16c(i64b),
        "fw1": np.ascontiguousarray(fw1), "fb1": np.ascontiguousarray(fb1),
        "fw2": fw2, "fb2": fb2,
    }
    x16 = x.astype(np.float16)
    return [
        {"x16": np.ascontiguousarray(x16[i * NPC : (i + 1) * NPC].T), **shared}
        for i in range(NC)
    ]


_CACHED = {}


def _get_program():
    if "nc" not in _CACHED:
        _CACHED["nc"] = build_program()
    return _CACHED["nc"]


def kernel(**inputs) -> np.ndarray:
    nc = _get_program()
    in_maps = make_in_maps(inputs)
    res = bass_utils.run_bass_kernel_spmd(nc, in_maps, list(range(NC)))
    z = np.concatenate(
        [res.results[i]["zout"].reshape(GPC, 1) for i in range(NC)], axis=0
    )
    return z.astype(np.float32)


# revision 39
# speedup vs baseline: 1.1682x; 1.1682x over previous
"""Trainium2 Bass kernel for nn_ModelClass_45724221833594 (gnn_message_passing).

Data-parallel across graphs: 64 graphs x 1024 nodes x 64 feats, kNN(k=6),
pre_nn MLP + global BatchNorm, 4x GeneralConv (add-aggr) + JK-cat pooling,
per-graph FFN. 8 NeuronCores, 8 graphs per core; only cross-core traffic is
a [64,2] AllReduce for the BatchNorm statistics.

v2: f16 everywhere on the tensor engine (fp32 runs at 1/4 rate), host-side
x transpose+cast (kills PE transposes), K=65 scores with mean-shifted
norm row (ranking-invariant), psum-direct top-8, per-graph-pair
score/conv software pipeline to keep all engines busy.

Self-contained: hardcodes all shapes; host side only slices/reshapes inputs.
"""

import sys

for _p in ("/opt/trn_rl_repo", "/root/.axon_site/_ro/trn_rl_repo"):
    if _p not in sys.path:
        sys.path.append(_p)

from contextlib import ExitStack

import numpy as np

import concourse.bass as bass
import concourse.tile as tile
from concourse import bacc, mybir
from concourse import bass_utils

F32 = mybir.dt.float32
F16 = mybir.dt.float16
U16 = mybir.dt.uint16
I16 = mybir.dt.int16
AF = mybir.ActivationFunctionType
OP = mybir.AluOpType

G, P, D, K = 64, 1024, 64, 6
NC = 8           # cores
GPC = G // NC    # graphs per core = 8
NPC = GPC * P    # nodes per core = 8192
CH = 512         # node chunk
NCH = NPC // CH  # 16 chunks per core
NTOT = G * P     # total nodes (for BN)


def build_program(num_cores=NC, sim_single=False):
    nc = bacc.Bacc("TRN2", target_bir_lowering=False, debug=False, num_devices=num_cores)

    din = {}
    def inp(name, shape, dtype=F32):
        din[name] = nc.dram_tensor(name, list(shape), dtype, kind="ExternalInput").ap()
        return din[name]

    x16_d = inp("x16", [D, NPC], F16)     # x^T, host-transposed + f16
    w1b_d = inp("w1b", [D + 1, D], F16)
    w2b_d = inp("w2b", [D + 1, D], F16)
    al_d = inp("alphas", [D, 6])          # a1, 1-a1, a2, 1-a2, aact, -aact (f32)
    gb_d = inp("gb", [D, 2])              # gamma, beta (f32)
    cw_d = inp("convw", [D, 4 * D], F16)  # W_l at cols l*64
    cb_d = inp("i64b", [D + 1, 4 * D], F16)  # per-layer [I64; 6*conv_b[l]]
    sqw_d = inp("sqw", [D + 1, 1], F16)   # rows 0-63 = -0.5, row 64 = +32
    fw1_d = inp("fw1", [D, 5 * 320])      # ffn_W1 [320,320] -> [64, L*320+j] (f32)
    fb1_d = inp("fb1", [1, 320])
    fw2_d = inp("fw2", [128, 3])          # ffn_W2 padded per 128-chunk
    fb2_d = inp("fb2", [1, 1])
    ones_d = inp("ones16", [1, NPC], F16)  # ones row (fast row init via DMA)

    z_d = nc.dram_tensor("zout", [1, GPC], F32, kind="ExternalOutput").ap()

    ccin = nc.dram_tensor("ccin", [D, 2], F32).ap()
    ccout = nc.dram_tensor("ccout", [D, 2], F32, addr_space="Shared").ap()

    with tile.TileContext(nc) as tc, ExitStack() as ctx:
        pool = lambda name, bufs: ctx.enter_context(tc.tile_pool(name=name, bufs=bufs))
        cpool = pool("consts", 1)
        ppoolA = ctx.enter_context(tc.tile_pool(name="pA", bufs=3, space="PSUM"))
        ppoolB = ctx.enter_context(tc.tile_pool(name="pB", bufs=1, space="PSUM"))
        spool = ctx.enter_context(tc.tile_pool(name="psco", bufs=2, space="PSUM"))

        # ---- constants to SBUF ----
        w1b = cpool.tile([D + 1, D], F16); nc.sync.dma_start(w1b[:], w1b_d[:])
        w2b = cpool.tile([D + 1, D], F16); nc.sync.dma_start(w2b[:], w2b_d[:])
        al = cpool.tile([D, 6], F32); nc.sync.dma_start(al[:], al_d[:])
        gb = cpool.tile([D, 2], F32); nc.sync.dma_start(gb[:], gb_d[:])
        cw = cpool.tile([D, 4 * D], F16); nc.sync.dma_start(cw[:], cw_d[:])
        i64b = cpool.tile([D + 1, 4 * D], F16); nc.sync.dma_start(i64b[:], cb_d[:])
        sqw = cpool.tile([D + 1, 1], F16); nc.sync.dma_start(sqw[:], sqw_d[:])
        fw1 = cpool.tile([D, 5 * 320], F32); nc.sync.dma_start(fw1[:], fw1_d[:])
        fb1 = cpool.tile([1, 320], F32); nc.sync.dma_start(fb1[:], fb1_d[:])
        fw2 = cpool.tile([128, 3], F32); nc.sync.dma_start(fw2[:], fw2_d[:])
        fb2 = cpool.tile([1, 1], F32); nc.sync.dma_start(fb2[:], fb2_d[:])
        ones6 = cpool.tile([128, K], F16); nc.vector.memset(ones6[:], 1.0)
        ones512 = cpool.tile([1, CH], F32); nc.vector.memset(ones512[:], 1.0)

        a1, a1m = al[:, 0:1], al[:, 1:2]
        a2, a2m = al[:, 2:3], al[:, 3:4]
        aact, naact = al[:, 4:5], al[:, 5:6]
        gamma, beta = gb[:, 0:1], gb[:, 1:2]

        # ---- big SBUF residents ----
        # X1: rows 0-63 x^T (f16), row 64 = 32 - |x|^2/2  (scores rhs)
        # X2: rows 0-63 x^T (f16), row 64 = ones          (scores lhsT, pre_nn rhs)
        X1 = cpool.tile([D + 1, NPC], F16)
        X2 = cpool.tile([D + 1, NPC], F16)
        nc.sync.dma_start(X2[D : D + 1, :], ones_d[:])
        h16 = cpool.tile([D + 1, NPC], F16)  # evolving h (f16); row 64 ones
        nc.sync.dma_start(h16[D : D + 1, :], ones_d[:])

        # accumulators for pooling (5 layers x 16 chunks) and BN stats
        pacc = cpool.tile([D, 5 * NCH], F32)
        hsum = cpool.tile([D, NCH], F32)
        hsq = cpool.tile([D, NCH], F32)

        # rotating f16 staging tiles with a preset ones row (K=65 bias trick)
        xsqs = []
        for _xi in range(3):
            _t = cpool.tile([D + 1, CH], F16, tag=f"xsq{_xi}")
            nc.sync.dma_start(_t[D : D + 1, :], ones_d[:, 0:CH])
            xsqs.append(_t)
        h1cs = []
        for _xi in range(3):
            _t = cpool.tile([D + 1, CH], F16, tag=f"h1c{_xi}")
            nc.sync.dma_start(_t[D : D + 1, :], ones_d[:, 0:CH])
            h1cs.append(_t)
        sqd = cpool.tile([D, CH], F16)  # scratch dst for BN sum-of-squares

        mv_p = pool("maxv", 4)
        adj_p = pool("adj", 1)
        adjt_p = pool("adjt", 6)
        u16_p = pool("u16", 2)
        sm_p = pool("small", 1)

        # DMA x^T into X1/X2 feature rows (4 chunks each for overlap)
        for q in range(4):
            qs = slice(q * 2048, (q + 1) * 2048)
            nc.sync.dma_start(X1[0:D, qs], x16_d[:, qs])
            nc.sync.dma_start(X2[0:D, qs], x16_d[:, qs])

        # ========= Phase 1+2 interleaved: norm rows, pre_nn, BN stats =========
        for c in range(NCH):
            sl = slice(c * CH, (c + 1) * CH)
            xsq = xsqs[c % 3]
            nc.vector.tensor_tensor(xsq[0:D, :], X1[0:D, sl], X1[0:D, sl], op=OP.mult)
            psq = ppoolA.tile([1, CH], F32, tag="pA")
            nc.tensor.matmul(psq[:], sqw[:], xsq[:])  # 32 - |x|^2/2
            nc.scalar.copy(X1[D : D + 1, sl], psq[:])
            p1 = ppoolA.tile([D, CH], F32, tag="pA")
            nc.tensor.matmul(p1[:], w1b[:], X2[:, sl])  # K=65 (bias via ones row)
            h1c = h1cs[c % 3]
            nc.scalar.activation(h1c[0:D, :], p1[:], AF.Prelu, alpha=a1)
            p2 = ppoolA.tile([D, CH], F32, tag="pA")
            nc.tensor.matmul(p2[:], w2b[:], h1c[:])  # K=65
            nc.scalar.activation(
                h16[0:D, sl], p2[:], AF.Prelu, alpha=a2,
                accum_out=hsum[:, c : c + 1],
            )
            # sum of squares for BN var (vector, to keep the scalar queue short)
            nc.vector.tensor_tensor(sqd[:], h16[0:D, sl], h16[0:D, sl], op=OP.mult)
            nc.vector.reduce_sum(hsq[:, c : c + 1], sqd[:], axis=mybir.AxisListType.X)

        # BN statistics allreduce (kick off early; math happens after scores g0/g1)
        st = sm_p.tile([D, 16], F32, tag="stats")
        nc.vector.reduce_sum(st[:, 0:1], hsum[:], axis=mybir.AxisListType.X)
        nc.vector.reduce_sum(st[:, 1:2], hsq[:], axis=mybir.AxisListType.X)
        ccr = st[:, 2:4]
        if sim_single:
            nc.scalar.copy(ccr, st[:, 0:2])
        else:
            nc.scalar.dma_start(ccin[:], st[:, 0:2])
            nc.gpsimd.collective_compute(
                "AllReduce",
                OP.add,
                replica_groups=[list(range(num_cores))],
                ins=[ccin[:]],
                outs=[ccout[:]],
            )
            nc.scalar.dma_start(ccr, ccout[:])

        # ================= kNN scores + top-8 + adjacency =================
        adjts = [None] * GPC

        def scores_graph(g):
            adj = adj_p.tile([128, GPC, P], F16)
            for b in range(8):  # query blocks of 128
                qsl = slice(g * P + b * 128, g * P + (b + 1) * 128)
                ps = spool.tile([128, P], F32, tag="psco")
                for hf in range(2):
                    csl = slice(g * P + hf * CH, g * P + (hf + 1) * CH)
                    nc.tensor.matmul(ps[:, hf * CH : (hf + 1) * CH], X2[:, qsl], X1[:, csl])
                vals = mv_p.tile([128, 8], F32, tag="mvals")
                idx = mv_p.tile([128, 8], U16, tag="midx")
                nc.vector.max(vals[:], ps[:])
                nc.vector.max_index(idx[:], vals[:], ps[:])
                nc.gpsimd.local_scatter(
                    adj[:, b, :],
                    ones6[:],
                    idx[:, 1 : K + 1].bitcast(I16),
                    channels=128,
                    num_elems=P,
                    num_idxs=K,
                )
            adjt = adjt_p.tile([128, 64, 128], F16)
            nc.sync.dma_start_transpose(adjt[:], adj[:])
            adjts[g] = adjt

        scores_graph(0)
        scores_graph(1)

        # ================= BN stats math + apply =================
        mean, msq = st[:, 4:5], st[:, 5:6]
        var, rv, rstd = st[:, 6:7], st[:, 7:8], st[:, 8:9]
        scale, shift, tmp = st[:, 9:10], st[:, 10:11], st[:, 11:12]
        ntot = NPC if sim_single else NTOT
        nc.vector.tensor_scalar_mul(mean, st[:, 2:3], 1.0 / ntot)
        nc.vector.tensor_scalar_mul(msq, st[:, 3:4], 1.0 / ntot)
        nc.vector.tensor_tensor(tmp, mean, mean, op=OP.mult)
        nc.vector.tensor_tensor(var, msq, tmp, op=OP.subtract)
        nc.vector.tensor_scalar_add(rv, var, 1e-5)
        nc.vector.reciprocal(rv, rv)
        nc.scalar.sqrt(rstd, rv)  # rstd = sqrt(1/(var+eps))
        nc.vector.tensor_tensor(scale, gamma, rstd, op=OP.mult)
        nc.vector.tensor_tensor(tmp, mean, scale, op=OP.mult)
        nc.vector.tensor_tensor(shift, beta, tmp, op=OP.subtract)
        for c in range(NCH):
            sl = slice(c * CH, (c + 1) * CH)
            nc.scalar.activation(
                h16[0:D, sl], h16[0:D, sl], AF.Identity,
                bias=shift, scale=scale, accum_out=pacc[:, c : c + 1],
            )

        for _g in range(2, 6):
            scores_graph(_g)

        # ================= conv layers =================
        def conv_graph_layer(g, l):
            av = adjts[g][:].rearrange("p (qb cb) f -> p cb qb f", cb=8)
            wl = cw[:, l * D : (l + 1) * D]
            pu = ppoolA.tile([128, 8 * D], F32, tag="pA")
            for sc in range(8):
                nc.tensor.matmul(
                    pu[:, sc * D : (sc + 1) * D],
                    h16[0:D, g * P + sc * 128 : g * P + (sc + 1) * 128],
                    wl,
                )
            u16 = u16_p.tile([128, GPC * D], F16)
            nc.scalar.copy(u16[:], pu[:])
            for dc in range(2):
                dsl = slice(g * P + dc * CH, g * P + (dc + 1) * CH)
                pm = ppoolA.tile([D, CH], F32, tag="pA")
                for sc in range(8):
                    nc.tensor.matmul(
                        pm[:],
                        u16[:, sc * D : (sc + 1) * D],
                        av[:, sc, 4 * dc : 4 * dc + 4, :],
                        start=(sc == 0),
                        stop=False,
                        skip_group_check=True,
                    )
                nc.tensor.matmul(
                    pm[:], i64b[:, l * D : (l + 1) * D], h16[:, dsl],
                    start=False, stop=True, skip_group_check=True,
                )
                col = (l + 1) * NCH + g * 2 + dc
                nc.scalar.activation(
                    h16[0:D, dsl], pm[:], AF.Prelu, alpha=aact,
                    accum_out=pacc[:, col : col + 1],
                )

        for gp in range(0, GPC, 2):
            if gp + 6 < GPC:
                scores_graph(gp + 6)
                scores_graph(gp + 7)
            for l in range(4):
                conv_graph_layer(gp, l)
                conv_graph_layer(gp + 1, l)

        # ================= pooling + FFN =================
        pooledT = sm_p.tile([D, 5 * GPC], F32, tag="pooled")  # col = L*8+g
        nc.vector.tensor_tensor(
            pooledT[:],
            pacc[:].rearrange("p (n two) -> p n two", two=2)[:, :, 0],
            pacc[:].rearrange("p (n two) -> p n two", two=2)[:, :, 1],
            op=OP.add,
        )
        z1L = sm_p.tile([128, 3 * GPC], F32, tag="z1l")
        for mc in range(3):
            msz = min(128, 320 - mc * 128)
            pz = ppoolB.tile([128, GPC], F32, tag="pB")
            for L in range(5):
                nc.tensor.matmul(
                    pz[0:msz, :],
                    fw1[:, L * 320 + mc * 128 : L * 320 + mc * 128 + msz],
                    pooledT[:, L * GPC : (L + 1) * GPC],
                    start=(L == 0),
                    stop=False,
                )
            nc.tensor.matmul(
                pz[0:msz, :], fb1[:, mc * 128 : mc * 128 + msz], ones512[:, 0:GPC],
                start=False, stop=True,
            )
            # LeakyReLU(0.01)
            nc.scalar.activation(
                z1L[0:msz, mc * GPC : (mc + 1) * GPC], pz[0:msz, :], AF.Prelu,
                alpha=0.01,
            )
        pzz = ppoolB.tile([1, GPC], F32, tag="pB")
        for mc in range(3):
            msz = min(128, 320 - mc * 128)
            nc.tensor.matmul(
                pzz[:], fw2[0:msz, mc : mc + 1], z1L[0:msz, mc * GPC : (mc + 1) * GPC],
                start=(mc == 0), stop=False,
            )
        nc.tensor.matmul(pzz[:], fb2[:], ones512[:, 0:GPC], start=False, stop=True)
        zsb = sm_p.tile([1, GPC], F32, tag="zsb")
        nc.scalar.copy(zsb[:], pzz[:])
        nc.sync.dma_start(z_d[:], zsb[:])

    nc.compile()
    return nc


def make_in_maps(inputs):
    """Host-side marshalling: slice + transpose x per core, lay out weights."""
    x = np.asarray(inputs["x"], dtype=np.float32)
    f32 = lambda a: np.asarray(a, dtype=np.float32)
    f16c = lambda a: np.ascontiguousarray(np.asarray(a, dtype=np.float16))

    w1b = np.concatenate([f32(inputs["pre_W1"]), f32(inputs["pre_b1"])[None, :]], 0)
    w2b = np.concatenate([f32(inputs["pre_W2"]), f32(inputs["pre_b2"])[None, :]], 0)
    a1 = f32(inputs["pre_a1"]); a2 = f32(inputs["pre_a2"]); aa = f32(inputs["act_a"])
    alphas = np.stack([a1, 1.0 - a1, a2, 1.0 - a2, aa, -aa], 1)
    gb = np.stack([f32(inputs["bn_gamma"]), f32(inputs["bn_beta"])], 1)
    convw = f32(inputs["conv_W"]).transpose(1, 0, 2).reshape(D, 4 * D)
    i64b = np.zeros((D + 1, 4 * D), np.float32)
    for l in range(4):
        i64b[0:D, l * D : (l + 1) * D] = np.eye(D, dtype=np.float32)
        i64b[D, l * D : (l + 1) * D] = 6.0 * f32(inputs["conv_b"])[l]
    fw1 = f32(inputs["ffn_W1"]).reshape(5, D, 320).transpose(1, 0, 2).reshape(D, 5 * 320)
    fb1 = f32(inputs["ffn_b1"]).reshape(1, 320)
    fw2 = np.zeros((128, 3), np.float32)
    w2flat = f32(inputs["ffn_W2"]).reshape(320)
    for mc in range(3):
        msz = min(128, 320 - mc * 128)
        fw2[:msz, mc] = w2flat[mc * 128 : mc * 128 + msz]
    fb2 = f32(inputs["ffn_b2"]).reshape(1, 1)
    sqw = np.zeros((D + 1, 1), np.float32)
    sqw[0:D, 0] = -0.5
    sqw[D, 0] = 32.0

    shared = {
        "ones16": np.ones((1, NPC), np.float16),
        "sqw": f16c(sqw),
        "w1b": f16c(w1b), "w2b": f16c(w2b),
        "alphas": np.ascontiguousarray(alphas), "gb": np.ascontiguousarray(gb),
        "convw": f16c(convw), "i64b": f16c(i64b),
        "fw1": np.ascontiguousarray(fw1), "fb1": np.ascontiguousarray(fb1),
        "fw2": fw2, "fb2": fb2,
    }
    x16 = x.astype(np.float16)
    return [
        {"x16": np.ascontiguousarray(x16[i * NPC : (i + 1) * NPC].T), **shared}
        for i in range(NC)
    ]


_CACHED = {}


def _get_program():
    if "nc" not in _CACHED:
        _CACHED["nc"] = build_program()
    return _CACHED["nc"]


def kernel(**inputs) -> np.ndarray:
    nc = _get_program()
    in_maps = make_in_maps(inputs)
    res = bass_utils.run_bass_kernel_spmd(nc, in_maps, list(range(NC)))
    z = np.concatenate(
        [res.results[i]["zout"].reshape(GPC, 1) for i in range(NC)], axis=0
    )
    return z.astype(np.float32)
